# revision 22
# baseline (speedup 1.0000x reference)
"""MAGNN intra-metapath attention aggregation on 8 Trainium2 NeuronCores.

Strategy: sort edges by destination node on the host, shard the node range
across the 8 cores (each core gets a contiguous node range and all of its
edges).  Each core's work is packed into B fixed blocks of (<=128 nodes,
<=T*128 edges); inside a block the segment softmax + weighted scatter-sum is
computed with selection-matrix matmuls on the PE array accumulating into one
PSUM tile [128 nodes, 256+8] (weighted-feature cols + per-head exp-sum
cols).  No segment max is needed: scores are O(1) so exp() cannot overflow,
and softmax is shift-invariant, so the result matches the reference to
rounding.  No cross-core collectives: node ranges are disjoint.

v3.1, tuned against the TRN2 cost model:
  - bf16 end-to-end with fp32 PSUM accumulation.
  - attn_r folded into feat on the host (feat' = feat * attn); the per-edge
    score is then a pure sum over d, computed as a log-tree of
    contiguous-half tensor_tensor adds (DVE 2x_1p fast mode; the tree tail
    + leaky-relu run on Pool).  The scale is undone after aggregation by one
    per-column 1/attn multiply (256 values once per 128 nodes, not per edge).
  - d-MAJOR columns (col = d*40 + t*8 + h) make the exp-weighting ONE flat
    tensor_tensor with a 3-dim broadcast (walrus rejects >3 canonical dims).
  - selection matrices are 0/1 data: they are built ON THE HOST and ride in
    the same DMA row as feat (bf16 exact), feeding matmul lhsT directly.
  - Act writes exp() straight into the vals tile; per-tile matmuls split
    feat-part/ex-part so every instruction carries <=1 sync wait.
  - software pipeline: score-chain + exp for block b+1 and post-chain for
    block b-1 are interleaved around block b's matmuls, so no engine waits
    on a same-iteration cross-engine product.
  - ELU via max(x, min(exp(x),1)-1); PSUM eviction on Act with bias=1e-30
    folding the empty-node clamp; DMAs batched 4 blocks per transfer
    (SWDGE fixed cost ~1us per DMA on Pool).
"""

import os
import sys

import numpy as np
import ml_dtypes

for _p in ("/opt/trn_rl_repo",):
    if _p not in sys.path and os.path.isdir(_p):
        sys.path.insert(0, _p)

H = 8
D = 32
HD = H * D          # 256
E = 500_000
N = 100_000
C = 8               # cores
P = 128             # partitions
T = 5               # edge tiles (of 128) per block
EPB = T * P         # 640 edge slots per block
B = 104             # blocks per core (sim'd max over seeds: 101), mult of G
G = 8               # blocks per DMA group
TH = T * H          # 40
FW = T * HD         # 1280 feat cols per block
SW = T * P          # 640 sel cols per block
FROW = FW + SW      # 1920 row elems per block (feat d-major + sel)
VW = HD + H         # 264 value columns
NEG_SLOPE = 0.01

BF16 = ml_dtypes.bfloat16

_CACHE = {}
LAST_RESULTS = None


def _build_bass():
    import concourse.bacc as bacc
    import concourse.mybir as mybir
    import concourse.tile as tile

    f32 = mybir.dt.float32
    bf16 = mybir.dt.bfloat16
    A = mybir.AluOpType
    Act = mybir.ActivationFunctionType
    nc = bacc.Bacc("TRN2", target_bir_lowering=False, debug=False)

    feat_h = nc.dram_tensor("feat", [(B // G) * P, G * FROW], bf16,
                            kind="ExternalInput")
    invw_h = nc.dram_tensor("invw", [P, HD], bf16, kind="ExternalInput")
    out_h = nc.dram_tensor("scratch", [B * P, HD], bf16, kind="ExternalOutput")

    feat_ap, out_ap = feat_h.ap(), out_h.ap()

    with tile.TileContext(nc) as tc:
        with (
            tc.tile_pool(name="const", bufs=1) as cpool,
            tc.tile_pool(name="feat", bufs=3) as fpool,
            tc.tile_pool(name="tmp", bufs=4) as tpool,
            tc.tile_pool(name="vals", bufs=4) as vpool,
            tc.tile_pool(name="small", bufs=10) as spool,
            tc.tile_pool(name="post", bufs=6) as opool,
            tc.tile_pool(name="outg", bufs=3) as ogpool,
            tc.tile_pool(name="psum", bufs=4, space="PSUM") as ppool,
        ):
            invw_t = cpool.tile([P, HD], bf16)
            nc.sync.dma_start(out=invw_t[:], in_=invw_h.ap())
            # pre-consume on DVE: no compute op may carry a 2nd DMA wait
            dummy_b = cpool.tile([P, 1], bf16)
            nc.vector.tensor_scalar_mul(out=dummy_b[:], in0=invw_t[:, 0:1], scalar1=1.0)

            featg = [None] * (B // G)
            NP = B // 2            # pair count; pair j = blocks (2j, 2j+1)
            PGR = G // 2           # pairs per DMA group
            vals = [None] * NP
            psum = [None] * B
            wsum = [None] * NP
            rec2p = [None] * NP
            outtp = [None] * NP
            expvp = [None] * NP
            outg = [None] * (B // G)

            def dma_in(g):
                featg[g] = fpool.tile([P, G * FROW], bf16, name="featg")
                nc.gpsimd.dma_start(
                    out=featg[g][:], in_=feat_ap[g * P:(g + 1) * P, :])

            def fgv(j):
                g, s = divmod(j, PGR)
                return featg[g][:].rearrange(
                    "p (blk c) -> p blk c", c=FROW)[:, 2 * s:2 * s + 2]

            def prelude(j):
                # score chain for pair j, one instruction per level
                fg = fgv(j)
                h1 = tpool.tile([P, 2 * 16 * TH], bf16, name="h1")
                nc.vector.tensor_tensor(
                    out=h1[:], in0=fg[:, :, 0:16 * TH],
                    in1=fg[:, :, 16 * TH:FW], op=A.add)
                h1v = h1[:].rearrange("p (blk c) -> p blk c", c=16 * TH)
                h2 = tpool.tile([P, 2 * 8 * TH], bf16, name="h2")
                nc.vector.tensor_tensor(
                    out=h2[:], in0=h1v[:, :, 0:8 * TH], in1=h1v[:, :, 8 * TH:],
                    op=A.add)
                h2v = h2[:].rearrange("p (blk c) -> p blk c", c=8 * TH)
                h3 = spool.tile([P, 2 * 4 * TH], bf16, name="h3")
                nc.gpsimd.tensor_tensor(
                    out=h3[:], in0=h2v[:, :, 0:4 * TH], in1=h2v[:, :, 4 * TH:],
                    op=A.add)
                h3v = h3[:].rearrange("p (blk c) -> p blk c", c=4 * TH)
                h4 = spool.tile([P, 2 * 2 * TH], bf16, name="h4")
                nc.gpsimd.tensor_tensor(
                    out=h4[:], in0=h3v[:, :, 0:2 * TH], in1=h3v[:, :, 2 * TH:],
                    op=A.add)
                h4v = h4[:].rearrange("p (blk c) -> p blk c", c=2 * TH)
                h5 = spool.tile([P, 2 * TH], bf16, name="h5")
                nc.gpsimd.tensor_tensor(
                    out=h5[:], in0=h4v[:, :, 0:TH], in1=h4v[:, :, TH:],
                    op=A.add)
                e = spool.tile([P, 2 * TH], bf16, name="e")
                nc.scalar.activation(
                    out=e[:], in_=h5[:], func=Act.Prelu, alpha=NEG_SLOPE)
                vals[j] = vpool.tile([P, 2 * (FW + TH)], bf16, name="vals")
                vv = vals[j][:].rearrange("p (blk c) -> p blk c", c=FW + TH)
                nc.scalar.activation(
                    out=vv[:, :, FW:],
                    in_=e[:].rearrange("p (blk c) -> p blk c", c=TH),
                    func=Act.Exp)

            def weigh_and_matmul(j):
                fg = fgv(j)
                vv = vals[j][:].rearrange("p (blk c) -> p blk c", c=FW + TH)
                wsum[j] = opool.tile([P, 2 * VW], bf16, name="wsum")
                for k in range(2):
                    b = 2 * j + k
                    nc.vector.tensor_tensor(
                        out=vv[:, k, 0:FW], in0=fg[:, k, 0:FW],
                        in1=vv[:, k, FW:][:, None, :].to_broadcast([P, D, TH]),
                        op=A.mult)
                    valtile = vv[:, k, 0:FW].rearrange(
                        "p (d t h) -> p t d h", t=T, h=H)
                    extile = vv[:, k, FW:].rearrange("p (t h) -> p t h", h=H)
                    ps = ppool.tile([P, VW], f32, space="PSUM", name="ps")
                    psum[b] = ps
                    for t in range(T):
                        nc.tensor.matmul(
                            out=ps[:, 0:HD],
                            lhsT=fg[:, k, FW + t * P:FW + (t + 1) * P],
                            rhs=valtile[:, t],
                            start=(t == 0), stop=(t == T - 1))
                    for t in range(T):
                        nc.tensor.matmul(
                            out=ps[:, HD:VW],
                            lhsT=fg[:, k, FW + t * P:FW + (t + 1) * P],
                            rhs=extile[:, t],
                            start=(t == 0), stop=(t == T - 1))
                    nc.scalar.activation(
                        out=wsum[j][:, k * VW:(k + 1) * VW], in_=ps[:],
                        func=Act.Copy)

            def post1(j):
                # den clamp + reciprocal (pair-wide) + 1/(attn*den) per block
                wv = wsum[j][:].rearrange("p (blk c) -> p blk c", c=VW)
                den = spool.tile([P, 2 * H], bf16, name="den")
                nc.vector.tensor_scalar_max(
                    out=den[:].rearrange("p (blk c) -> p blk c", c=H),
                    in0=wv[:, :, HD:VW], scalar1=1e-30)
                rec = spool.tile([P, 2 * H], bf16, name="rec")
                with nc.allow_low_precision(
                        reason="attn weights tolerate bf16 reciprocal"):
                    nc.vector.reciprocal(out=rec[:], in_=den[:])
                rec2p[j] = opool.tile([P, 2 * HD], bf16, name="rec2p")
                for k in range(2):
                    nc.gpsimd.tensor_tensor(
                        out=rec2p[j][:, k * HD:(k + 1) * HD].rearrange(
                            "p (d h) -> p d h", h=H),
                        in0=invw_t[:].rearrange("p (d h) -> p d h", h=H),
                        in1=rec[:, k * H:(k + 1) * H][:, None, :].to_broadcast(
                            [P, D, H]),
                        op=A.mult)

            def post2(j):
                wv = wsum[j][:].rearrange("p (blk c) -> p blk c", c=VW)
                outtp[j] = opool.tile([P, 2 * HD], bf16, name="outtp")
                for k in range(2):
                    nc.vector.tensor_tensor(
                        out=outtp[j][:, k * HD:(k + 1) * HD],
                        in0=wv[:, k, 0:HD],
                        in1=rec2p[j][:, k * HD:(k + 1) * HD], op=A.mult)
                expvp[j] = opool.tile([P, 2 * HD], bf16, name="expvp")
                nc.scalar.activation(
                    out=expvp[j][:], in_=outtp[j][:], func=Act.Exp)

            def finish(j):
                # ELU(x) = max(x, min(exp(x),1)-1), pair-wide
                g, s = divmod(j, PGR)
                em = opool.tile([P, 2 * HD], bf16, name="em")
                nc.vector.tensor_scalar(
                    out=em[:], in0=expvp[j][:], scalar1=1.0, scalar2=-1.0,
                    op0=A.min, op1=A.add)
                if s == 0:
                    outg[g] = ogpool.tile([P, G * HD], bf16, name="outg")
                nc.vector.tensor_tensor(
                    out=outg[g][:, 2 * s * HD:(2 * s + 2) * HD],
                    in0=outtp[j][:], in1=em[:], op=A.max)
                if s == PGR - 1:
                    nc.gpsimd.dma_start(
                        out=out_ap[g * G * P:(g + 1) * G * P, :].rearrange(
                            "(blk p) c -> p blk c", p=P),
                        in_=outg[g][:].rearrange("p (blk c) -> p blk c", c=HD))

            # software pipeline over pairs:
            #   finish(i-4) | post1(i-2) | post2(i-3) | prelude(i+1) | weigh(i)
            dma_in(0)
            dma_in(1)
            prelude(0)
            NPG = B // G
            for i in range(NP + 4):
                if i % PGR == 0 and i // PGR + 2 < NPG:
                    dma_in(i // PGR + 2)
                if 0 <= i - 4 < NP:
                    finish(i - 4)
                if 0 <= i - 2 < NP:
                    post1(i - 2)
                if 0 <= i - 3 < NP:
                    post2(i - 3)
                if i + 1 < NP:
                    prelude(i + 1)
                if i < NP:
                    weigh_and_matmul(i)
    nc.compile()
    return nc


def pack_inputs(feat0, attn_r, dst_idx):
    """Sort by dst, shard nodes across cores, pack blocks.

    Block row (per partition p): 1280 bf16 feat cols d-major
    (col = d*40 + t*8 + h) with attn folded in, then 5*128 bf16 one-hot
    selection cols; 4 blocks per DMA row group.
    Returns (in_maps, meta); meta[c] = per-block (n0, n1) node ranges."""
    attn_flat = attn_r.reshape(H, D).astype(np.float32)        # [h, d]
    order = np.argsort(dst_idx, kind="stable")
    dst_s = dst_idx[order]
    feat_s = (feat0[order].reshape(-1, H, D) * attn_flat[None]).astype(BF16)

    deg = np.bincount(dst_s, minlength=N)
    cum = np.concatenate([[0], np.cumsum(deg)])

    in_maps = []
    meta = []
    invw = np.ascontiguousarray(
        (1.0 / attn_flat.T).reshape(1, HD)).repeat(P, axis=0).astype(BF16)

    for c in range(C):
        n0c, n1c = c * N // C, (c + 1) * N // C
        blocks = []
        n = n0c
        while n < n1c:
            hi = int(np.searchsorted(cum, cum[n] + EPB, side="right")) - 1
            nn = min(hi, n + P, n1c)
            assert nn > n, f"node {n} has degree {deg[n]} > {EPB}"
            blocks.append((n, nn))
            n = nn
        assert len(blocks) <= B, f"core {c} needs {len(blocks)} blocks > {B}"
        while len(blocks) < B:
            blocks.append((n1c, n1c))  # empty tail blocks

        feat_blk = np.zeros((B, P, T, H, D), dtype=BF16)
        sel_blk = np.zeros((B, P, T, P), dtype=BF16)
        for b, (bn0, bn1) in enumerate(blocks):
            e0, e1 = int(cum[bn0]), int(cum[bn1])
            ne = e1 - e0
            if ne == 0:
                continue
            dr = (dst_s[e0:e1] - bn0).astype(np.int64)
            t_idx, p_idx = np.divmod(np.arange(ne), P)
            feat_blk[b, p_idx, t_idx] = feat_s[e0:e1].reshape(-1, H, D)
            sel_blk[b, p_idx, t_idx, dr] = 1.0
        # d-major: [P, T, H, D] -> [P, D, T, H]
        feat_dev = feat_blk.transpose(0, 1, 4, 2, 3).reshape(B, P, FW)
        row = np.concatenate(
            [feat_dev, sel_blk.reshape(B, P, SW)], axis=2)     # [B, P, 1920]
        grouped = np.ascontiguousarray(
            row.reshape(B // G, G, P, FROW).transpose(0, 2, 1, 3)
        ).reshape((B // G) * P, G * FROW)
        in_maps.append({
            "feat": grouped,
            "invw": invw,
        })
        meta.append(blocks)
    return in_maps, meta


def kernel(feat0, attn_r, dst_idx, num_dst):
    global LAST_RESULTS
    feat0 = np.asarray(feat0, dtype=np.float32)
    attn_r = np.asarray(attn_r, dtype=np.float32)
    dst_idx = np.asarray(dst_idx).astype(np.int64)
    num_dst = int(num_dst)
    assert feat0.shape == (E, HD) and num_dst == N

    in_maps, meta = pack_inputs(feat0, attn_r, dst_idx)

    if "nc" not in _CACHE:
        _CACHE["nc"] = _build_bass()
    nc = _CACHE["nc"]

    from concourse import bass_utils
    res = bass_utils.run_bass_kernel_spmd(
        nc, in_maps, core_ids=list(range(C)),
        trace=bool(int(os.environ.get("KBASS_TRACE", "0"))),
    )
    LAST_RESULTS = res

    out = np.zeros((N, HD), dtype=np.float32)
    for c in range(C):
        scratch = res.results[c]["scratch"].astype(np.float32)
        # columns are [d, h]; restore [h, d]
        scratch = scratch.reshape(B, P, D, H).transpose(0, 1, 3, 2)
        scratch = scratch.reshape(B, P, HD)
        for b, (bn0, bn1) in enumerate(meta[c]):
            if bn1 > bn0:
                out[bn0:bn1] = scratch[b, : bn1 - bn0]
    return out
